# revision 8
# baseline (speedup 1.0000x reference)
"""CapsuleLayer kernel for 8 trn2 NeuronCores.

Math (from the reference):
    c        = softmax(bias[0,:,:,0,0], axis=1)            # [I, J]
    s[b,j,d] = sum_{i,p} x[b,i,p] * W[i,j,p,d] * c[i,j]    # [B, J, D]
    out      = squash(s, axis=-1)

Folding c into W gives one big matmul
    s = X @ Wc,  X: [B, K], Wc: [K, N],  K = I*P = 32768, N = J*D = 1024.

Sharding: split the contraction dim K across the 8 cores (each core reads a
distinct 1/8 slice of W, so W is read exactly once fleet-wide — the memory
roofline optimum). Each core computes a partial [B, N] sum; the host adds
the 8 partials (1 MB total) and applies the tiny squash.

Precision (MODE) — this problem family gates at rel_err < 2e-2:
  "fp16x1" — x and Wc cast to fp16 (Wc pre-scaled by 2^8 so it stays
            normal-range). Products accumulate in the PE's fp32 PSUM;
            measured 3.0e-4 relative — 67x inside the gate.

Layout: one input tensor per core, K-tile-major: each 128-row K-tile packs
[x cols | W cols] contiguously, so chunked DMA streams feed everything at
~420 GB/s (HWDGE rows of 4-8KB). Trace-driven structure:
  - chunks alternate between the Sync (Q1) and Scalar (Q10) HWDGE queues
    so the 16 DMA engines see two descriptor streams and idle less;
  - per-bank PSUM tiles + per-bank SBUF out tiles remove false
    write-write deps so bank-0 eviction overlaps bank-1's last matmuls;
  - evictions avoid the Scalar ACTIVATE path (no ACT_TABLE_LOAD);
  - tail chunks taper to 1 K-tile so the PE finishes ~0.4us after the
    last weight byte;
  - dummy matmuls at both ends hold the PE-activity-driven clock (HAM)
    at 2.4 GHz through the DMA ramp-in and the drain/teardown epilogue
    (otherwise the core halves its clock ~2us after the PE idles and the
    epilogue crawls).
"""

import ml_dtypes
import numpy as np

import concourse.mybir as mybir
import concourse.tile as tile
from concourse import bacc
from concourse import bass_utils as _bass_utils
from concourse.bass_utils import run_bass_kernel_spmd

# The backend (walrus) resets the full 256-sem file at end-of-program as
# ~250 per-sem writes split evenly across the 5 engines — a fixed ~5us
# tail on every execution (Tensor's ~50 clears at ~115ns apiece are the
# critical path). The new codegen backend has a grouped-reset instruction.
_WALRUS_EXTRA_FLAGS = ["--enable-new-backend"]


def _install_walrus_flags():
    if getattr(_bass_utils.run_command, "_capsule_patched", False):
        return
    orig = _bass_utils.run_command

    def patched(cmd, *a, **kw):
        if (
            isinstance(cmd, list)
            and cmd
            and "walrus_driver" in str(cmd[0])
            and any("codegen" in str(c) for c in cmd[:4])
        ):
            cmd = list(cmd) + _WALRUS_EXTRA_FLAGS
        return orig(cmd, *a, **kw)

    patched._capsule_patched = True
    _bass_utils.run_command = patched


_install_walrus_flags()

MODE = "fp16x1"        # "fp16x1" | "fp16" | "bf16x3"

# Problem shapes (hardcoded per contract).
B, I, P, J, D = 64, 2048, 16, 32, 32
K = I * P            # 32768 contraction
N = J * D            # 1024 output features
N_CORES = 8
K_CORE = K // N_CORES  # 4096 contraction rows per core
KT = 128               # K-tile (partition dim of one matmul)
NKT = K_CORE // KT     # 32 K-tiles per core
# DMA chunk sizes (in K-tiles), summing to NKT. Bulk chunks of 4 give
# 8.7KB HWDGE rows (best per-engine bandwidth); the 2,1 tail keeps the
# last-arrival -> last-matmul gap small.
CHUNKS = [2, 3, 4, 4, 4, 4, 4, 4, 2, 1]
NB = N // 512          # PSUM-bank-sized slices of N (bank = 512 fp32)
N_WARM = 8             # dummy matmuls to lift the PE HAM clock gate
N_TAIL = 12            # dummy matmuls to hold the clock through teardown
W_SCALE = 256.0        # exact power-of-2 lift keeping fp16(Wc) normal

if MODE == "fp16x1":
    NXP = 1            # x parts: single fp16
    NWP = 1            # w parts: single fp16
    NP_DTYPE = np.float16
    MM_DTYPE = mybir.dt.float16
    TERMS = [(0, 0)]
elif MODE == "fp16":
    NXP = 2            # x parts (hi, lo)
    NWP = 1
    NP_DTYPE = np.float16
    MM_DTYPE = mybir.dt.float16
    TERMS = [(0, 0), (1, 0)]
else:
    NXP = 2
    NWP = 2
    NP_DTYPE = ml_dtypes.bfloat16
    MM_DTYPE = mybir.dt.bfloat16
    TERMS = [(0, 0), (0, 1), (1, 0)]  # drops the lo@lo term

TC = NXP * B + NWP * N  # packed columns per K-tile

_NC_CACHE = None


def _build_nc():
    """Per-core program: out[B,N] accumulated over 32 K-tiles in PSUM."""
    nc = bacc.Bacc(trn_type="TRN2", target_bir_lowering=False, debug=False)
    f16 = mybir.dt.float16

    wx = nc.dram_tensor("wx", [KT, NKT * TC], MM_DTYPE, kind="ExternalInput")
    out = nc.dram_tensor("out", [B, N], f16, kind="ExternalOutput")

    assert sum(CHUNKS) == NKT
    n_small = sum(1 for s in CHUNKS if s <= 2)
    n_big = sum(1 for s in CHUNKS if s > 2)
    with tile.TileContext(nc) as tc:
        with (
            tc.tile_pool(name="cpool", bufs=1) as cpool,
            # One buffer per chunk (no slot reuse) so every chunk DMA can be
            # in flight at once; small/big pools so slots aren't all padded
            # to the largest chunk (SBUF budget).
            tc.tile_pool(name="wsmall", bufs=max(n_small, 1)) as wsmall,
            tc.tile_pool(name="wbig", bufs=max(n_big, 1)) as wbig,
            tc.tile_pool(name="opool", bufs=1) as opool,
            tc.tile_pool(name="pspool", bufs=1, space="PSUM") as pspool,
        ):
            # HAM warm-up: PE must stay busy ~3.4us to reach 2.4 GHz. These
            # dummies depend only on a memset tile, so they run during the
            # first chunk's DMA flight.
            warm = cpool.tile([KT, 512], MM_DTYPE)
            nc.vector.memset(warm[:], 1.0)
            warm_ps = pspool.tile([B, 512], mybir.dt.float32)
            for _ in range(N_WARM):
                nc.tensor.matmul(
                    warm_ps[:], warm[:, 0:B], warm[:], start=True, stop=True
                )

            # Per-bank PSUM tiles: separate dep chains so bank 0's eviction
            # only waits on bank 0's final matmul.
            ps = [
                pspool.tile([B, 512], mybir.dt.float32, name=f"ps{nb}")
                for nb in range(NB)
            ]

            def tile_views(w_sb, tl):
                base = tl * TC
                xp = [
                    w_sb[:, base + k * B : base + (k + 1) * B]
                    for k in range(NXP)
                ]
                wcol = [base + NXP * B + k * N for k in range(NWP)]
                return xp, wcol

            t = 0
            col = 0
            for ci, csz in enumerate(CHUNKS):
                pool = wsmall if csz <= 2 else wbig
                w_sb = pool.tile([KT, csz * TC], MM_DTYPE)
                # Single HWDGE queue (Sync=Q1): one dense descriptor stream
                # keeps all 16 DMA engines saturated. (Alternating chunks
                # with the Scalar queue was measured slower: Q10 has ~3us
                # ring-ramp latency and the staggered completions starve
                # the PE into a mid-stream HAM downclock.)
                nc.sync.dma_start(w_sb[:], wx.ap()[:, col : col + csz * TC])
                col += csz * TC
                if ci < len(CHUNKS) - 1:
                    # lhsT-major groups pair weight loads.
                    for tl in range(csz):
                        xp, wcol = tile_views(w_sb, tl)
                        for xi, wi in TERMS:
                            for nb in range(NB):
                                nc.tensor.matmul(
                                    ps[nb][:],
                                    xp[xi],
                                    w_sb[:, wcol[wi] + nb * 512 : wcol[wi] + (nb + 1) * 512],
                                    start=(t + tl == 0 and (xi, wi) == TERMS[0]),
                                    stop=False,
                                )
                else:
                    # Last chunk bank-major so bank 0 gets its stop first
                    # and its eviction overlaps bank 1's tail.
                    for nb in range(NB):
                        for tl in range(csz):
                            xp, wcol = tile_views(w_sb, tl)
                            for ti, (xi, wi) in enumerate(TERMS):
                                nc.tensor.matmul(
                                    ps[nb][:],
                                    xp[xi],
                                    w_sb[:, wcol[wi] + nb * 512 : wcol[wi] + (nb + 1) * 512],
                                    start=False,
                                    stop=(tl == csz - 1 and ti == len(TERMS) - 1),
                                )
                t += csz

            # Per-bank eviction on disjoint engine chains (GpSimd cannot
            # read PSUM on TRN2). fp32 PSUM -> fp16 SBUF -> fp16 DRAM; the
            # host sums the 8 per-core partials in fp32.
            o0 = opool.tile([B, 512], f16)
            o1 = opool.tile([B, 512], f16)
            nc.scalar.copy(o0[:], ps[0][:])
            nc.scalar.dma_start(out.ap()[:, 0:512], o0[:])
            nc.vector.tensor_copy(o1[:], ps[1][:])
            nc.sync.dma_start(out.ap()[:, 512:1024], o1[:])

            # Hold the HAM clock at 2.4 GHz through the output store and
            # the drain/sem-reset teardown (the clock halves ~2us after the
            # PE goes idle, which doubles the ~3us fixed postamble cost).
            # Reading o0 pins these after the bank-0 eviction — without a
            # dependency the tile scheduler hoists them into mid-stream
            # idle slots where they do nothing for the tail clock.
            for _ in range(N_TAIL):
                nc.tensor.matmul(
                    warm_ps[:], o0[:, 0:B], warm[0:B, :], start=True, stop=True
                )
    # Run Bacc's compile pipeline (wait legalization, register allocation).
    # run_bass_via_pjrt serializes nc.m as-is and never finalizes.
    nc.finalize()
    return nc


def _get_nc():
    global _NC_CACHE
    if _NC_CACHE is None:
        _NC_CACHE = _build_nc()
    return _NC_CACHE


def _prepare_in_maps(inputs: np.ndarray, W: np.ndarray, bias: np.ndarray):
    """Fold softmax(bias) into W, split precision, pack K-tile-major."""
    x = np.asarray(inputs, dtype=np.float32)
    Wf = np.asarray(W, dtype=np.float32)
    b = np.asarray(bias, dtype=np.float32)[0, :, :, 0, 0]          # [I, J]

    # softmax over J per input capsule i (fp32, matches jax.nn.softmax).
    m = b.max(axis=1, keepdims=True)
    e = np.exp(b - m)
    c = e / e.sum(axis=1, keepdims=True)                            # [I, J]

    # Wc[(i,p),(j,d)] = W[i,j,p,d] * c[i,j]  ->  [K, N]
    wc = (Wf.transpose(0, 2, 1, 3) * c[:, None, :, None]).reshape(K, N)
    xT = np.ascontiguousarray(x.reshape(B, K).T)                    # [K, B]

    xh = xT.astype(NP_DTYPE)
    if NXP == 1:
        xparts = [xh]
    else:
        xl = (xT - xh.astype(np.float32)).astype(NP_DTYPE)
        xparts = [xh, xl]
    if NWP == 1:
        wparts = [(wc * np.float32(W_SCALE)).astype(NP_DTYPE)]
    else:
        wh = wc.astype(NP_DTYPE)
        wl = (wc - wh.astype(np.float32)).astype(NP_DTYPE)
        wparts = [wh, wl]

    packed = np.empty((K, TC), dtype=NP_DTYPE)
    for k in range(NXP):
        packed[:, k * B : (k + 1) * B] = xparts[k]
    for k in range(NWP):
        packed[:, NXP * B + k * N : NXP * B + (k + 1) * N] = wparts[k]

    in_maps = []
    for cid in range(N_CORES):
        sl = slice(cid * K_CORE, (cid + 1) * K_CORE)
        # K-tile-major packing: [NKT, KT, TC] -> [KT, NKT*TC]
        core = np.ascontiguousarray(
            packed[sl].reshape(NKT, KT, TC).swapaxes(0, 1).reshape(KT, NKT * TC)
        )
        in_maps.append({"wx": core})
    return in_maps


def _squash(s: np.ndarray) -> np.ndarray:
    s2 = np.sum(np.square(s), axis=-1, keepdims=True, dtype=np.float32)
    scale = s2 / (1.0 + s2) / np.sqrt(s2)
    return (scale * s).astype(np.float32)


def run(inputs, W, bias, **spmd_kwargs):
    """Full pipeline; returns (output, BassKernelResults)."""
    in_maps = _prepare_in_maps(inputs, W, bias)
    try:
        res = run_bass_kernel_spmd(
            _get_nc(), in_maps, core_ids=list(range(N_CORES)), **spmd_kwargs
        )
    except Exception:
        # A crashed prior process can leave a core wedged
        # (NRT_EXEC_UNIT_UNRECOVERABLE); one retry clears it.
        import time
        time.sleep(2.0)
        res = run_bass_kernel_spmd(
            _get_nc(), in_maps, core_ids=list(range(N_CORES)), **spmd_kwargs
        )
    s = np.zeros((B, N), dtype=np.float32)
    for r in res.results:
        s += np.asarray(r["out"], dtype=np.float32)
    if NWP == 1:
        s /= np.float32(W_SCALE)
    out = _squash(s.reshape(B, J, D))
    return out, res


def kernel(inputs, W, bias):
    out, _ = run(inputs, W, bias)
    return out


# revision 12
# speedup vs baseline: 1.1479x; 1.1479x over previous
"""CapsuleLayer kernel for 8 trn2 NeuronCores.

Math (from the reference):
    c        = softmax(bias[0,:,:,0,0], axis=1)            # [I, J]
    s[b,j,d] = sum_{i,p} x[b,i,p] * W[i,j,p,d] * c[i,j]    # [B, J, D]
    out      = squash(s, axis=-1)

Folding c into W gives one big matmul
    s = X @ Wc,  X: [B, K], Wc: [K, N],  K = I*P = 32768, N = J*D = 1024.

Sharding: split the contraction dim K across the 8 cores (each core reads a
distinct 1/8 slice of W, so W is read exactly once fleet-wide — the memory
roofline optimum). Each core computes a partial [B, N] sum; the host adds
the 8 partials (1 MB total) and applies the tiny squash.

Precision (MODE) — this problem family gates at rel_err < 2e-2:
  "fp16x1" — x and Wc cast to fp16 (Wc pre-scaled by 2^8 so it stays
            normal-range). Products accumulate in the PE's fp32 PSUM;
            measured 3.0e-4 relative — 67x inside the gate.

Layout: one input tensor per core, K-tile-major: each 128-row K-tile packs
[x cols | W cols] contiguously, so chunked DMA streams feed everything at
~420 GB/s (HWDGE rows of 4-8KB). Trace-driven structure:
  - chunks alternate between the Sync (Q1) and Scalar (Q10) HWDGE queues
    so the 16 DMA engines see two descriptor streams and idle less;
  - per-bank PSUM tiles + per-bank SBUF out tiles remove false
    write-write deps so bank-0 eviction overlaps bank-1's last matmuls;
  - evictions avoid the Scalar ACTIVATE path (no ACT_TABLE_LOAD);
  - tail chunks taper to 1 K-tile so the PE finishes ~0.4us after the
    last weight byte;
  - dummy matmuls at both ends hold the PE-activity-driven clock (HAM)
    at 2.4 GHz through the DMA ramp-in and the drain/teardown epilogue
    (otherwise the core halves its clock ~2us after the PE idles and the
    epilogue crawls).
"""

import ml_dtypes
import numpy as np

import concourse.mybir as mybir
import concourse.tile as tile
from concourse import bacc
from concourse import bass_utils as _bass_utils
from concourse.bass_utils import run_bass_kernel_spmd

# The backend (walrus) resets the full 256-sem file at end-of-program as
# ~250 per-sem writes split evenly across the 5 engines — a fixed ~5us
# tail on every execution (Tensor's ~50 clears at ~115ns apiece are the
# critical path; sequencers are not HAM-gated, so this cost is clock-
# independent). Measured: --max-sem-num and --enable-new-backend do not
# shrink it and slow the body; leave walrus flags alone.


class _SlimTileContext(tile.TileContext):
    """TileContext with a lighter exit: the stock epilogue emits
    drain + barrier + (dma_reset + sem_clear) + barrier, but walrus's own
    end-of-program teardown already range-resets the whole sem file and
    barriers all engines before the final notify. Keep the drain (output
    DMA completion) and one barrier; keep the gpsimd dma_reset/sem_clear
    (cheap range ops, needed for HWDGE queue state on re-execution); skip
    the second all-engine barrier — the race it guards against (an engine
    re-entering a cleared sem range) cannot happen when the program ends
    right after, and walrus's final barrier still fences the notify."""

    def _drain_and_barrier(self, tick_clock, wait_clock):
        from concourse.vector_clock import ScopedClock

        drain_inst = self.nc.sync.drain()
        wait_clock.add_sem_waits(
            drain_inst.ins, ScopedClock({None: tick_clock.global_clock})
        )
        self.nc.all_engine_barrier()
        popped = self.nc._tile_sem_poison_stack.pop()
        assert popped is self._sem_poison
        self.nc.clear_and_free_semaphores(list(self.sems.allocated().values()))

MODE = "fp16x1"        # "fp16x1" | "fp16" | "bf16x3"

# Problem shapes (hardcoded per contract).
B, I, P, J, D = 64, 2048, 16, 32, 32
K = I * P            # 32768 contraction
N = J * D            # 1024 output features
N_CORES = 8
K_CORE = K // N_CORES  # 4096 contraction rows per core
KT = 128               # K-tile (partition dim of one matmul)
NKT = K_CORE // KT     # 32 K-tiles per core
# DMA chunk sizes (in K-tiles), summing to NKT. Bulk chunks of 4 give
# 8.7KB HWDGE rows (best per-engine bandwidth); the 2,1 tail keeps the
# last-arrival -> last-matmul gap small.
CHUNKS = [2, 3, 4, 4, 4, 4, 4, 4, 2, 1]
NB = N // 512          # PSUM-bank-sized slices of N (bank = 512 fp32)
N_WARM = 8             # dummy matmuls to lift the PE HAM clock gate
W_SCALE = 256.0        # exact power-of-2 lift keeping fp16(Wc) normal

if MODE == "fp16x1":
    NXP = 1            # x parts: single fp16
    NWP = 1            # w parts: single fp16
    NP_DTYPE = np.float16
    MM_DTYPE = mybir.dt.float16
    TERMS = [(0, 0)]
elif MODE == "fp16":
    NXP = 2            # x parts (hi, lo)
    NWP = 1
    NP_DTYPE = np.float16
    MM_DTYPE = mybir.dt.float16
    TERMS = [(0, 0), (1, 0)]
else:
    NXP = 2
    NWP = 2
    NP_DTYPE = ml_dtypes.bfloat16
    MM_DTYPE = mybir.dt.bfloat16
    TERMS = [(0, 0), (0, 1), (1, 0)]  # drops the lo@lo term

TC = NXP * B + NWP * N  # packed columns per K-tile

_NC_CACHE = None


def _build_nc():
    """Per-core program: out[B,N] accumulated over 32 K-tiles in PSUM."""
    nc = bacc.Bacc(trn_type="TRN2", target_bir_lowering=False, debug=False)
    f16 = mybir.dt.float16

    wx = nc.dram_tensor("wx", [KT, NKT * TC], MM_DTYPE, kind="ExternalInput")
    out = nc.dram_tensor("out", [B, N], f16, kind="ExternalOutput")

    assert sum(CHUNKS) == NKT
    n_small = sum(1 for s in CHUNKS if s <= 2)
    n_big = sum(1 for s in CHUNKS if s > 2)
    with _SlimTileContext(nc) as tc:
        with (
            tc.tile_pool(name="cpool", bufs=1) as cpool,
            # One buffer per chunk (no slot reuse) so every chunk DMA can be
            # in flight at once; small/big pools so slots aren't all padded
            # to the largest chunk (SBUF budget).
            tc.tile_pool(name="wsmall", bufs=max(n_small, 1)) as wsmall,
            tc.tile_pool(name="wbig", bufs=max(n_big, 1)) as wbig,
            tc.tile_pool(name="opool", bufs=1) as opool,
            tc.tile_pool(name="pspool", bufs=1, space="PSUM") as pspool,
        ):
            # HAM warm-up: PE must stay busy ~3.4us to reach 2.4 GHz. These
            # dummies depend only on a memset tile, so they run during the
            # first chunk's DMA flight.
            warm = cpool.tile([KT, 512], MM_DTYPE)
            nc.vector.memset(warm[:], 1.0)
            warm_ps = pspool.tile([B, 512], mybir.dt.float32)
            for _ in range(N_WARM):
                nc.tensor.matmul(
                    warm_ps[:], warm[:, 0:B], warm[:], start=True, stop=True
                )

            # Per-bank PSUM tiles: separate dep chains so bank 0's eviction
            # only waits on bank 0's final matmul.
            ps = [
                pspool.tile([B, 512], mybir.dt.float32, name=f"ps{nb}")
                for nb in range(NB)
            ]

            def tile_views(w_sb, tl):
                base = tl * TC
                xp = [
                    w_sb[:, base + k * B : base + (k + 1) * B]
                    for k in range(NXP)
                ]
                wcol = [base + NXP * B + k * N for k in range(NWP)]
                return xp, wcol

            t = 0
            col = 0
            for ci, csz in enumerate(CHUNKS):
                pool = wsmall if csz <= 2 else wbig
                w_sb = pool.tile([KT, csz * TC], MM_DTYPE)
                # Single HWDGE queue (Sync=Q1): one dense descriptor stream
                # keeps all 16 DMA engines saturated. (Alternating chunks
                # with the Scalar queue was measured slower: Q10 has ~3us
                # ring-ramp latency and the staggered completions starve
                # the PE into a mid-stream HAM downclock.)
                nc.sync.dma_start(w_sb[:], wx.ap()[:, col : col + csz * TC])
                col += csz * TC
                if ci < len(CHUNKS) - 1:
                    # lhsT-major groups pair weight loads.
                    for tl in range(csz):
                        xp, wcol = tile_views(w_sb, tl)
                        for xi, wi in TERMS:
                            for nb in range(NB):
                                nc.tensor.matmul(
                                    ps[nb][:],
                                    xp[xi],
                                    w_sb[:, wcol[wi] + nb * 512 : wcol[wi] + (nb + 1) * 512],
                                    start=(t + tl == 0 and (xi, wi) == TERMS[0]),
                                    stop=False,
                                )
                else:
                    # Last chunk bank-major so bank 0 gets its stop first
                    # and its eviction overlaps bank 1's tail.
                    for nb in range(NB):
                        for tl in range(csz):
                            xp, wcol = tile_views(w_sb, tl)
                            for ti, (xi, wi) in enumerate(TERMS):
                                nc.tensor.matmul(
                                    ps[nb][:],
                                    xp[xi],
                                    w_sb[:, wcol[wi] + nb * 512 : wcol[wi] + (nb + 1) * 512],
                                    start=False,
                                    stop=(tl == csz - 1 and ti == len(TERMS) - 1),
                                )
                t += csz

            # Per-bank eviction on disjoint engine chains (GpSimd cannot
            # read PSUM on TRN2). fp32 PSUM -> fp16 SBUF -> fp16 DRAM; the
            # host sums the 8 per-core partials in fp32.
            o0 = opool.tile([B, 512], f16)
            o1 = opool.tile([B, 512], f16)
            nc.scalar.copy(o0[:], ps[0][:])
            nc.scalar.dma_start(out.ap()[:, 0:512], o0[:])
            nc.vector.tensor_copy(o1[:], ps[1][:])
            nc.sync.dma_start(out.ap()[:, 512:1024], o1[:])
            # (No tail dummy matmuls: HAM only gates the PE array clock,
            # not the sequencers that run the fixed sem-reset teardown —
            # tail dummies just delay Tensor's barrier arrival 1:1.)
    # Run Bacc's compile pipeline (wait legalization, register allocation).
    # run_bass_via_pjrt serializes nc.m as-is and never finalizes.
    nc.finalize()
    return nc


def _get_nc():
    global _NC_CACHE
    if _NC_CACHE is None:
        _NC_CACHE = _build_nc()
    return _NC_CACHE


def _prepare_in_maps(inputs: np.ndarray, W: np.ndarray, bias: np.ndarray):
    """Fold softmax(bias) into W, split precision, pack K-tile-major."""
    x = np.asarray(inputs, dtype=np.float32)
    Wf = np.asarray(W, dtype=np.float32)
    b = np.asarray(bias, dtype=np.float32)[0, :, :, 0, 0]          # [I, J]

    # softmax over J per input capsule i (fp32, matches jax.nn.softmax).
    m = b.max(axis=1, keepdims=True)
    e = np.exp(b - m)
    c = e / e.sum(axis=1, keepdims=True)                            # [I, J]

    # Wc[(i,p),(j,d)] = W[i,j,p,d] * c[i,j]  ->  [K, N]
    wc = (Wf.transpose(0, 2, 1, 3) * c[:, None, :, None]).reshape(K, N)
    xT = np.ascontiguousarray(x.reshape(B, K).T)                    # [K, B]

    xh = xT.astype(NP_DTYPE)
    if NXP == 1:
        xparts = [xh]
    else:
        xl = (xT - xh.astype(np.float32)).astype(NP_DTYPE)
        xparts = [xh, xl]
    if NWP == 1:
        wparts = [(wc * np.float32(W_SCALE)).astype(NP_DTYPE)]
    else:
        wh = wc.astype(NP_DTYPE)
        wl = (wc - wh.astype(np.float32)).astype(NP_DTYPE)
        wparts = [wh, wl]

    packed = np.empty((K, TC), dtype=NP_DTYPE)
    for k in range(NXP):
        packed[:, k * B : (k + 1) * B] = xparts[k]
    for k in range(NWP):
        packed[:, NXP * B + k * N : NXP * B + (k + 1) * N] = wparts[k]

    in_maps = []
    for cid in range(N_CORES):
        sl = slice(cid * K_CORE, (cid + 1) * K_CORE)
        # K-tile-major packing: [NKT, KT, TC] -> [KT, NKT*TC]
        core = np.ascontiguousarray(
            packed[sl].reshape(NKT, KT, TC).swapaxes(0, 1).reshape(KT, NKT * TC)
        )
        in_maps.append({"wx": core})
    return in_maps


def _squash(s: np.ndarray) -> np.ndarray:
    s2 = np.sum(np.square(s), axis=-1, keepdims=True, dtype=np.float32)
    scale = s2 / (1.0 + s2) / np.sqrt(s2)
    return (scale * s).astype(np.float32)


def run(inputs, W, bias, **spmd_kwargs):
    """Full pipeline; returns (output, BassKernelResults)."""
    in_maps = _prepare_in_maps(inputs, W, bias)
    try:
        res = run_bass_kernel_spmd(
            _get_nc(), in_maps, core_ids=list(range(N_CORES)), **spmd_kwargs
        )
    except Exception:
        # A crashed prior process can leave a core wedged
        # (NRT_EXEC_UNIT_UNRECOVERABLE); one retry clears it.
        import time
        time.sleep(2.0)
        res = run_bass_kernel_spmd(
            _get_nc(), in_maps, core_ids=list(range(N_CORES)), **spmd_kwargs
        )
    s = np.zeros((B, N), dtype=np.float32)
    for r in res.results:
        s += np.asarray(r["out"], dtype=np.float32)
    if NWP == 1:
        s /= np.float32(W_SCALE)
    out = _squash(s.reshape(B, J, D))
    return out, res


def kernel(inputs, W, bias):
    out, _ = run(inputs, W, bias)
    return out
